# revision 20
# baseline (speedup 1.0000x reference)
"""Trainium2 Bass kernel for nn_Encoder_MLP (embedding gather + sum + 2-layer MLP tail).

Reference computation:
    x = where(gate_seq < 0, A, gate_seq)            # [B, T]   (inputs never negative)
    Wr = W1.reshape(T, V, HID)
    h  = Wr[arange(T)[None,:], x].sum(axis=1) + b1  # [B, HID]  gather B*T rows, sum over T
    h  = relu(h); h = relu(h @ W2 + b2); out = h @ W3 + b3

Sharding (8 cores): data-parallel over the batch axis, W1 fully replicated
(bf16, 512 MB/core in HBM). Core m owns batches [8m, 8m+8) and gathers all
T=256 positions for them. NO collective: the T-sharded variant's [64,256]
ReduceScatter cost 95+us of barrier/ncfw/RS on a 115-137us exec (the
collectives-init barrier absorbs cross-core NEFF launch skew), while this
batch-parallel kernel runs in ~39-41us without it.

Gather-phase resource balance: dma_gather idx are int16 (<=32768 addressable
elements per call). Per-call measured costs: desc-gen ~1.4us fixed + ~8ns/idx
(serial per SWDGE queue, 4 queues), queue DMA channel ~35 GB/s. All-single-row
calls (32 x 64 idx, 1 MB) are desc-bound (~15.2us); all-2-row-block calls
(16 x 128 idx, 2 MB) are DMA-bound (~14.4us). This kernel MIXES them to
balance both streams: positions 0..127 as 8 PAIR windows (16 positions,
idx = (j*4096+x)>>1 = j*2048+(x>>1) addresses [32768, 512]-viewed 2-row
blocks, 128 idx/call) and positions 128..255 as 16 SINGLE windows
(8 positions, row idx = j*4096+x, 64 idx/call). Per queue: 2 pair + 4 single
calls, pairs issued first = ~12.4us desc + ~11us DMA, phase ~13.3us.

Reduce (all hidden under the gather cadence):
- pair windows: wanted half of each block picked by parity = x & 1 via
    h += bmask^T . SUM_w even_w  +  SUM_w om_w^T . (odd_w - even_w)
  where bmask[p,b] = (p%8 == b), om_w[p,:] = bmask[p,:] * parity[p,w]:
  8 DVE subs + 8 om matmuls + a PROGRESSIVE even-sum chain (7 DVE adds,
  each issued right after its windows land) + 1 bmask matmul. parity comes
  from gate_T, a second value-independent host permutation of gate_seq.
- single windows: row i lands on partition i (i = j*8 + b), one accumulating
  bmask[0:64] matmul per window right after its gather.
All matmuls accumulate one f32 PSUM [8, 256] group.

Index layout: idx list position i lives at idx_tile[i%16, base + i//16]
(16-partition wrap, replicated x8 for the 8 Q7 cores); i = j*8 + b so
partition p holds batch p%8. The j*2048 / j*4096 rebase is an inline const
added on device; the pair-half x>>1 is a device tensor_scalar shift; the
host only permutes/retypes gate_seq (value-independent layout marshaling).

Tail MLP (per-core [8,256], no second transpose): PE-transpose h ->
relu(+b1T) -> 4 matmuls with W2 128x128 chunks as lhsT (output already
transposed) -> relu(+b2T) -> 2 matmuls + rank-1 bias matmul -> [8,256].
Host concatenates the per-core outputs.
"""

import sys

import numpy as np

if "/opt/trn_rl_repo" not in sys.path:
    sys.path.insert(0, "/opt/trn_rl_repo")

B = 64
T = 256
V = 4096
HID = 256
OUT = 256
NCORES = 8
BPC = B // NCORES          # batches per core = 8
NQ = 4                     # SWDGE queues

NP = 8                     # pair windows (16 positions each): positions [0, 128)
PWIN_POS = 16
PWIN_ROWS = PWIN_POS * V   # 65536 rows = 32768 2-row blocks
P_NIDX = BPC * PWIN_POS    # 128 idx per pair call
P_IDXC = P_NIDX // 16      # 8 idx cols per pair window

NS = 16                    # single windows (8 positions each): positions [128, 256)
SBASE_POS = NP * PWIN_POS  # 128
SWIN_POS = 8
SWIN_ROWS = SWIN_POS * V   # 32768 rows
S_NIDX = BPC * SWIN_POS    # 64 idx per single call
S_IDXC = S_NIDX // 16      # 4 idx cols per single window

PCOLS = NP * P_IDXC        # 64 idx columns for the pair half
SCOLS = NS * S_IDXC        # 64 idx columns for the single half

_CACHE = {}


def _host_consts():
    import ml_dtypes

    p = np.arange(128)[:, None]
    # pair half: i = (col % 8)*16 + p%16; j = i//8 -> rebase j*2048
    colp = np.arange(PCOLS)[None, :]
    ip = (colp % P_IDXC) * 16 + (p % 16)
    ub_p = (ip // BPC) * (V // 2)
    # single half: i = (col % 4)*16 + p%16; j = i//8 -> rebase j*4096
    cols = np.arange(SCOLS)[None, :]
    i_s = (cols % S_IDXC) * 16 + (p % 16)
    ub_s = (i_s // BPC) * V
    ubias = np.concatenate(
        [np.broadcast_to(ub_p, (128, PCOLS)), np.broadcast_to(ub_s, (128, SCOLS))],
        axis=1,
    ).astype(np.int16)
    # bmask[p, b] = 1 if p % 8 == b
    bmask = (np.arange(128)[:, None] % BPC == np.arange(BPC)[None, :]).astype(
        ml_dtypes.bfloat16
    )
    eye8 = np.eye(8, dtype=np.float32)
    return np.ascontiguousarray(ubias), np.ascontiguousarray(bmask), eye8


def _build_nc():
    import concourse.bacc as bacc
    import concourse.mybir as mybir
    import concourse.tile as tile

    f32 = mybir.dt.float32
    bf16 = mybir.dt.bfloat16
    i16 = mybir.dt.int16
    Relu = mybir.ActivationFunctionType.Relu
    add = mybir.AluOpType.add
    sub = mybir.AluOpType.subtract
    mult = mybir.AluOpType.mult
    shr = mybir.AluOpType.logical_shift_right
    band = mybir.AluOpType.bitwise_and

    ubias_np, bmask_np, eye8_np = _host_consts()

    nc = bacc.Bacc(
        "TRN2",
        target_bir_lowering=False,
        debug=False,
        num_devices=NCORES,
        num_swdge_queues=NQ,
    )

    gate_prep_d = nc.dram_tensor(
        "gate_prep", [128, PCOLS + SCOLS], i16, kind="ExternalInput"
    )
    gate_t_d = nc.dram_tensor("gate_t", [128, NP], i16, kind="ExternalInput")
    w1_d = nc.dram_tensor("w1", [T * V, HID], bf16, kind="ExternalInput")
    w2lh_d = nc.dram_tensor("w2lh", [128, 2, 2, 128], bf16, kind="ExternalInput")
    w3_d = nc.dram_tensor("w3", [HID, OUT], bf16, kind="ExternalInput")
    b1_d = nc.dram_tensor("b1t", [128, 2], f32, kind="ExternalInput")
    b2_d = nc.dram_tensor("b2t", [128, 2], f32, kind="ExternalInput")
    b3_d = nc.dram_tensor("b3", [1, OUT], bf16, kind="ExternalInput")
    out_d = nc.dram_tensor("out", [BPC, OUT], f32, kind="ExternalOutput")

    ubias_d = nc.inline_tensor(ubias_np, name="ubias_const")
    bmask_d = nc.inline_tensor(bmask_np, name="bmask_const")
    eye_d = nc.inline_tensor(eye8_np, name="eye_const")

    # Issue the mlp ucode library load before any Tile-scheduled work so the
    # ~10us Q7 library fetch overlaps the NEFF prologue instead of stalling
    # the first dma_gather until ~16us.
    from concourse import library_config

    nc.gpsimd.load_library(library_config.mlp)

    with tile.TileContext(nc) as tc:
        with (
            tc.tile_pool(name="const", bufs=1) as const,
            tc.tile_pool(name="gat", bufs=1) as gat,
            tc.tile_pool(name="work", bufs=2) as work,
            tc.tile_pool(name="psum", bufs=1, space="PSUM") as psum,
        ):
            # ---- critical path: indices (pair half: x>>1 + j*2048; single: x + j*4096)
            gp = const.tile([128, PCOLS + SCOLS], i16, tag="gp")
            nc.sync.dma_start(gp[:], gate_prep_d[:])
            ub = const.tile([128, PCOLS + SCOLS], i16, tag="ub")
            nc.sync.dma_start(ub[:], ubias_d[:])
            idx = const.tile([128, PCOLS + SCOLS], i16, tag="idx")
            nc.vector.tensor_scalar(idx[:, 0:PCOLS], gp[:, 0:PCOLS], 1, None, shr)
            nc.vector.tensor_tensor(idx[:, 0:PCOLS], idx[:, 0:PCOLS], ub[:, 0:PCOLS], add)
            nc.vector.tensor_tensor(idx[:, PCOLS:], gp[:, PCOLS:], ub[:, PCOLS:], add)

            # ---- parity masks (DVE; deps: gate_t DMA only) ----
            gt = const.tile([128, NP], i16, tag="gt")
            nc.sync.dma_start(gt[:], gate_t_d[:])
            bmask_sb = const.tile([128, BPC], bf16, tag="bmask")
            nc.scalar.dma_start(bmask_sb[:], bmask_d[:])
            par_i = const.tile([128, NP], i16, tag="par_i")
            nc.vector.tensor_scalar(par_i[:], gt[:], 1, None, band)
            parf = const.tile([128, NP], f32, tag="parf")
            nc.vector.tensor_copy(parf[:], par_i[:])
            om = const.tile([128, NP, BPC], bf16, tag="om")
            for w in range(NP):
                nc.vector.tensor_scalar(
                    om[:, w, :], bmask_sb[:], parf[:, w : w + 1], None, mult
                )

            # ---- consts / weights preload ----
            eye_sb = const.tile([8, 8], f32, tag="eye")
            nc.scalar.dma_start(eye_sb[:], eye_d[:])
            w2lh_sb = const.tile([128, 2, 2, 128], bf16, tag="w2lh")
            nc.scalar.dma_start(w2lh_sb[:], w2lh_d[:])
            w3_sb = const.tile([128, 2, OUT], bf16, tag="w3")
            nc.scalar.dma_start(w3_sb[:], w3_d[:, :].rearrange("(k p) n -> p k n", p=128))
            b1_sb = const.tile([128, 2], f32, tag="b1")
            nc.scalar.dma_start(b1_sb[:], b1_d[:])
            b2_sb = const.tile([128, 2], f32, tag="b2")
            nc.scalar.dma_start(b2_sb[:], b2_d[:])
            b3_sb = const.tile([1, OUT], bf16, tag="b3")
            nc.scalar.dma_start(b3_sb[:], b3_d[:])
            ones8 = const.tile([1, BPC], bf16, tag="ones8")
            nc.vector.memset(ones8[:], 1.0)

            # ---- gathers: scheduled queues + fused reduce ----
            # Cold first-calls cost ~1.5-1.7us each with dispatch depth ~2, so
            # each queue warms up on a cheap SINGLE call (pulls the last
            # queue's start ~2us earlier), the DMA-heavy PAIR calls go right
            # after the warm-up (maximum drain window), and the load stays
            # uniform at 6 calls (1s + 2p + 3s) per queue.
            gp_tiles = []
            for w in range(NP):
                g = gat.tile([128, 1, 2 * HID], bf16, tag=f"gpair{w}")
                gp_tiles.append(g)
            gs_tiles = []
            for u in range(NS):
                g = gat.tile([128, 1, HID], bf16, tag=f"gsin{u}")
                gs_tiles.append(g)

            def ev(w):
                return gp_tiles[w][:, 0, 0:HID]

            def od(w):
                return gp_tiles[w][:, 0, HID : 2 * HID]

            QSCHED = [
                [("s", 0), ("p", 0), ("p", 4), ("s", 4), ("s", 8), ("s", 12)],
                [("s", 1), ("p", 1), ("p", 5), ("s", 5), ("s", 9), ("s", 13)],
                [("s", 2), ("p", 2), ("p", 6), ("s", 6), ("s", 10), ("s", 14)],
                [("s", 3), ("p", 3), ("p", 7), ("s", 7), ("s", 11), ("s", 15)],
            ]
            d = work.tile([128, NP, HID], bf16, tag="d")
            lv = work.tile([128, NP // 2, HID], bf16, tag="lv")
            psum_part = psum.tile([BPC, HID], f32, tag="part")
            first_mm = True
            pair_seen = set()
            for pos in range(6):
                for q in range(NQ):
                    kind, w = QSCHED[q][pos]
                    if kind == "p":
                        win = w1_d[w * PWIN_ROWS : (w + 1) * PWIN_ROWS, :].rearrange(
                            "(a two) n -> a (two n)", two=2
                        )
                        nc.gpsimd.dma_gather(
                            gp_tiles[w][:],
                            win,
                            idx[:, w * P_IDXC : (w + 1) * P_IDXC],
                            P_NIDX,
                            P_NIDX,
                            2 * HID,
                            queue_num=q,
                        )
                        nc.vector.tensor_tensor(d[:, w, :], od(w), ev(w), sub)
                        nc.tensor.matmul(
                            psum_part[:], om[:, w, :], d[:, w, :],
                            start=first_mm, stop=False,
                        )
                        first_mm = False
                        pair_seen.add(w)
                        if (w ^ 1) in pair_seen:
                            lo = min(w, w ^ 1)
                            nc.vector.tensor_tensor(
                                lv[:, lo // 2, :], ev(lo), ev(lo + 1), add
                            )
                    else:
                        base = SBASE_POS * V + w * SWIN_ROWS
                        nc.gpsimd.dma_gather(
                            gs_tiles[w][:],
                            w1_d[base : base + SWIN_ROWS, :],
                            idx[:, PCOLS + w * S_IDXC : PCOLS + (w + 1) * S_IDXC],
                            S_NIDX,
                            S_NIDX,
                            HID,
                            queue_num=q,
                        )
                        nc.tensor.matmul(
                            psum_part[:],
                            bmask_sb[0:64, :],
                            gs_tiles[w][0:64, 0, :],
                            start=first_mm,
                            stop=False,
                        )
                        first_mm = False
            # progressive even-sum chain (each add gated only by its leaves)
            acc = work.tile([128, HID], bf16, tag="acc")
            nc.vector.tensor_tensor(acc[:], lv[:, 0, :], lv[:, 1, :], add)
            nc.vector.tensor_tensor(acc[:], acc[:], lv[:, 2, :], add)
            nc.vector.tensor_tensor(acc[:], acc[:], lv[:, 3, :], add)
            # even-sum contribution closes the PSUM group
            nc.tensor.matmul(psum_part[:], bmask_sb[:], acc[:], start=False, stop=True)

            h_sb = work.tile([BPC, HID], f32, tag="h")
            nc.vector.tensor_copy(h_sb[:], psum_part[:])

            # ---- tail MLP on [8, 256] shard ----
            hTr = work.tile([128, 2, BPC], bf16, tag="hTr")
            for c in range(2):
                p_hT = psum.tile([128, BPC], f32, tag=f"p_hT{c}")
                nc.tensor.transpose(
                    p_hT[:], h_sb[:, c * 128 : (c + 1) * 128], eye_sb[:]
                )
                nc.scalar.activation(
                    hTr[:, c, :], p_hT[:], Relu, bias=b1_sb[:, c : c + 1]
                )
            h2Tr = work.tile([128, 2, BPC], bf16, tag="h2Tr")
            for c in range(2):
                p_h2T = psum.tile([128, BPC], f32, tag=f"p_h2T{c}")
                nc.tensor.matmul(
                    p_h2T[:], w2lh_sb[:, 0, c, :], hTr[:, 0, :], start=True, stop=False
                )
                nc.tensor.matmul(
                    p_h2T[:], w2lh_sb[:, 1, c, :], hTr[:, 1, :], start=False, stop=True
                )
                nc.scalar.activation(
                    h2Tr[:, c, :], p_h2T[:], Relu, bias=b2_sb[:, c : c + 1]
                )
            # bias matmul first: it has no data deps, so it runs early and the
            # last data matmul closes the group off the critical path
            p_o = psum.tile([BPC, OUT], f32, tag="p_o")
            nc.tensor.matmul(p_o[:], ones8[:], b3_sb[:], start=True, stop=False)
            nc.tensor.matmul(p_o[:], h2Tr[:, 0, :], w3_sb[:, 0, :], start=False, stop=False)
            nc.tensor.matmul(p_o[:], h2Tr[:, 1, :], w3_sb[:, 1, :], start=False, stop=True)
            out_sb = work.tile([BPC, OUT], f32, tag="out_sb")
            nc.vector.tensor_copy(out_sb[:], p_o[:])
            nc.sync.dma_start(out_d[:], out_sb[:])

    nc.compile()
    return nc


def get_nc():
    if "nc" not in _CACHE:
        _CACHE["nc"] = _build_nc()
    return _CACHE["nc"]


def make_in_maps(gate_seq, W1, b1, W2, b2, W3, b3):
    """Shard/marshal the full inputs into per-core input maps (values untouched:
    pure slicing, transposition, retyping and tiling)."""
    gate_seq = np.asarray(gate_seq)
    import ml_dtypes

    W1 = np.ascontiguousarray(np.asarray(W1).astype(ml_dtypes.bfloat16))
    W2 = np.asarray(W2, dtype=np.float32)
    W3 = np.ascontiguousarray(np.asarray(W3).astype(ml_dtypes.bfloat16))
    b1 = np.asarray(b1, dtype=np.float32)
    b2 = np.asarray(b2, dtype=np.float32)
    b3 = np.asarray(b3, dtype=np.float32)

    # W2 chunked for lhsT use: w2lh[p, kc, nc, f] = W2[kc*128 + p, nc*128 + f]
    w2lh = np.ascontiguousarray(
        W2.reshape(2, 128, 2, 128).transpose(1, 0, 2, 3).astype(ml_dtypes.bfloat16)
    )
    b1t = np.ascontiguousarray(b1.reshape(2, 128).T)  # b1t[p, c] = b1[c*128 + p]
    b2t = np.ascontiguousarray(b2.reshape(2, 128).T)
    b3r = np.ascontiguousarray(b3[None, :].astype(ml_dtypes.bfloat16))

    # index-layout permutations (see module docstring)
    p16 = np.arange(16)[:, None]
    # pair half
    colp = np.arange(PCOLS)[None, :]
    ip = (colp % P_IDXC) * 16 + p16
    bp = ip % BPC
    tp = (colp // P_IDXC) * PWIN_POS + ip // BPC
    # single half
    cols = np.arange(SCOLS)[None, :]
    i_s = (cols % S_IDXC) * 16 + p16
    bs = i_s % BPC
    ts = SBASE_POS + (cols // S_IDXC) * SWIN_POS + i_s // BPC
    b_idx = np.concatenate([bp, bs], axis=1)
    t_idx = np.concatenate([np.broadcast_to(tp, bp.shape), np.broadcast_to(ts, bs.shape)], axis=1)
    # gate_T[p, w] = gate_seq[8m + p%8, w*16 + p//8]  (pair windows only)
    pp = np.arange(128)[:, None]
    ww = np.arange(NP)[None, :]
    bt_idx = np.broadcast_to(pp % BPC, (128, NP))
    tt_idx = ww * PWIN_POS + pp // BPC

    in_maps = []
    for m in range(NCORES):
        gs = gate_seq[m * BPC : (m + 1) * BPC, :]    # [8, 256]
        A = gs[b_idx, t_idx].astype(np.int16)        # [16, PCOLS+SCOLS]
        gate_prep = np.ascontiguousarray(np.tile(A, (8, 1)))
        gate_t = np.ascontiguousarray(gs[bt_idx, tt_idx].astype(np.int16))  # [128, NP]
        in_maps.append(
            {
                "gate_prep": gate_prep,
                "gate_t": gate_t,
                "w1": W1,
                "w2lh": w2lh,
                "w3": W3,
                "b1t": b1t,
                "b2t": b2t,
                "b3": b3r,
            }
        )
    return in_maps


def run(inputs, trace=False, **spmd_kwargs):
    from concourse.bass_utils import run_bass_kernel_spmd

    nc = get_nc()
    in_maps = make_in_maps(**inputs)
    res = run_bass_kernel_spmd(
        nc, in_maps, core_ids=list(range(NCORES)), trace=trace, **spmd_kwargs
    )
    out = np.concatenate([r["out"] for r in res.results], axis=0)
    return out, res


def kernel(**inputs) -> np.ndarray:
    out, _ = run(inputs, trace=False)
    return out
